# revision 1
# baseline (speedup 1.0000x reference)
"""MoD router kernel for 8 Trainium2 NeuronCores.

Full inputs: x [4, 8192, 1024] f32, w_router [1024] f32, w_block [1024, 1024] f32.
out[b, l] = gelu_tanh(x[b, l] @ w_block) if l in topk(x[b] @ w_router, k=6144)
            else x[b, l]
(top-k membership is all that matters: the reference scatters processed rows
back to their own positions.)

Sharding: core c <- batch row c//2, contiguous half c%2 of L (4096 tokens).
Per core:
  - stream x tiles [128, 1024], keep resident in SBUF
  - scores on DVE (x * w_router_bcast, row-reduce)
  - PE-transpose x tiles -> f32r, 16 f32r matmuls vs w_block (K=1024), gelu on
    ACT, bulk-write gelu(xW) for ALL tokens to out
  - pairwise AllGather of scores (16KB) -> full-row scores
  - 40-iter bisection (all lanes replicated) for the k-th largest score
  - fixup: indirect-scatter resident x rows over out where score < thr
    (selected rows get OOB offsets and are skipped)
"""
import sys

if "/opt/trn_rl_repo" not in sys.path:
    sys.path.insert(0, "/opt/trn_rl_repo")

from contextlib import ExitStack

import numpy as np

import concourse.bass as bass
import concourse.tile as tile
from concourse import bacc, mybir
from concourse.bass_utils import run_bass_kernel_spmd
from concourse.masks import make_identity

dt = mybir.dt
AF = mybir.ActivationFunctionType
ALU = mybir.AluOpType

P = 128
B, L, D = 4, 8192, 1024
TLOC = L // 2          # tokens per core
NT = TLOC // P         # 32 t-tiles per core
DC = D // P            # 8 contraction chunks
K_SEL = int(L * 0.75)  # 6144
N_BISECT = 40
SCORE_BOUND = 64.0

_cached = {}


def build_kernel():
    nc = bacc.Bacc("TRN2", target_bir_lowering=False, debug=False, num_devices=8)
    x_d = nc.dram_tensor("x", [TLOC, D], dt.float32, kind="ExternalInput")
    wr_d = nc.dram_tensor("w_router", [D], dt.float32, kind="ExternalInput")
    wb_d = nc.dram_tensor("w_block", [D, D], dt.float32r, kind="ExternalInput")
    out_d = nc.dram_tensor("out", [TLOC, D], dt.float32, kind="ExternalOutput")
    sc_in = nc.dram_tensor("sc_in", [TLOC], dt.float32, kind="Internal")
    sc_out = nc.dram_tensor("sc_out", [L], dt.float32, kind="Internal")

    with tile.TileContext(nc) as tc, ExitStack() as ctx:
        const = ctx.enter_context(tc.tile_pool(name="const", bufs=1))
        xpool = ctx.enter_context(tc.tile_pool(name="xn", bufs=NT))
        wpool = ctx.enter_context(tc.tile_pool(name="wb", bufs=1))
        xtp = ctx.enter_context(tc.tile_pool(name="xt", bufs=2))
        yp = ctx.enter_context(tc.tile_pool(name="y", bufs=2))
        smalls = ctx.enter_context(tc.tile_pool(name="smalls", bufs=1))
        psx = ctx.enter_context(tc.tile_pool(name="psx", bufs=2, space="PSUM"))
        psy = ctx.enter_context(tc.tile_pool(name="psy", bufs=2, space="PSUM"))
        psm = ctx.enter_context(tc.tile_pool(name="psm", bufs=1, space="PSUM"))

        # ---- constants ----
        ident = const.tile([P, P], dt.float32)
        make_identity(nc, ident[:])
        ones_pp = const.tile([P, P], dt.float32)
        nc.vector.memset(ones_pp[:], 1.0)
        ones_row = const.tile([1, P], dt.float32)
        nc.vector.memset(ones_row[:], 1.0)

        w_sb = wpool.tile([P, DC, D], dt.float32r)
        nc.sync.dma_start(w_sb[:], wb_d.ap().rearrange("(a p) e -> p a e", p=P))

        wr_sb = const.tile([1, D], dt.float32)
        nc.sync.dma_start(wr_sb[:], wr_d.ap())
        # broadcast w_router over all partitions via K=1 matmul
        w_rep = const.tile([P, D], dt.float32)
        pm = psm.tile([P, D], dt.float32, tag="psm")
        for h in range(2):
            sl = slice(h * 512, (h + 1) * 512)
            nc.tensor.matmul(pm[:, sl], ones_row[:], wr_sb[:, sl],
                             start=True, stop=True)
        nc.vector.tensor_copy(w_rep[:], pm[:])

        # ---- score / bisection tiles ----
        scores_loc = smalls.tile([P, NT], dt.float32)
        scores_full = smalls.tile([P, 2 * NT], dt.float32)
        ge = smalls.tile([P, 2 * NT], dt.float32)
        cnt = smalls.tile([P, 1], dt.float32)
        lo = smalls.tile([P, 1], dt.float32)
        hi = smalls.tile([P, 1], dt.float32)
        mid = smalls.tile([P, 1], dt.float32)
        pred = smalls.tile([P, 1], dt.float32)
        npred = smalls.tile([P, 1], dt.float32)
        t1 = smalls.tile([P, 1], dt.float32)
        msel = smalls.tile([P, NT], dt.float32)
        ids = smalls.tile([P, NT], dt.int32)
        big_f = smalls.tile([P, NT], dt.float32)
        big_i = smalls.tile([P, NT], dt.int32)
        offs = smalls.tile([P, NT], dt.int32)
        tmp = smalls.tile([P, D], dt.float32)

        # ---- main loop: load, score, transpose, GEMM, gelu, write ----
        xns = []
        for i in range(NT):
            xn = xpool.tile([P, D], dt.float32, tag="xn")
            xns.append(xn)
            with nc.named_scope("load"):
                nc.sync.dma_start(xn[:], x_d.ap()[i * P:(i + 1) * P, :])
            with nc.named_scope("scores"):
                nc.vector.tensor_tensor(out=tmp[:], in0=xn[:], in1=w_rep[:],
                                        op=ALU.mult)
                nc.vector.reduce_sum(scores_loc[:, i:i + 1], tmp[:],
                                     axis=mybir.AxisListType.X)
            xt = xtp.tile([P, DC, P], dt.float32r, tag="xt")
            px = psx.tile([P, DC, P], dt.float32, tag="psx")
            with nc.named_scope("xpose"):
                for c in range(DC):
                    nc.tensor.transpose(px[:, c, :], xn[:, c * P:(c + 1) * P],
                                        ident[:])
                nc.any.tensor_copy(out=xt[:], in_=px[:])
            y = yp.tile([P, D], dt.float32, tag="y")
            for h in range(2):
                py = psy.tile([P, 512], dt.float32, tag="psy")
                with nc.named_scope("gemm"):
                    for c in range(DC):
                        nc.tensor.matmul(
                            py[:], xt[:, c, :],
                            w_sb[:, c, h * 512:(h + 1) * 512],
                            start=(c == 0), stop=(c == DC - 1))
                with nc.named_scope("gelu"):
                    nc.scalar.activation(y[:, h * 512:(h + 1) * 512], py[:],
                                         AF.Gelu_apprx_tanh)
            with nc.named_scope("store"):
                nc.sync.dma_start(out_d.ap()[i * P:(i + 1) * P, :], y[:])

        # ---- threshold side-chain (high priority so it never starves) ----
        with tc.high_priority():
            with nc.named_scope("coll"):
                nc.sync.dma_start(sc_in.ap(), scores_loc[:])
                nc.gpsimd.collective_compute(
                    "AllGather", ALU.bypass,
                    ins=[sc_in.ap()], outs=[sc_out.ap()],
                    replica_groups=[[0, 1], [2, 3], [4, 5], [6, 7]])
                nc.sync.dma_start(scores_full[:], sc_out.ap())
            with nc.named_scope("bisect"):
                nc.vector.memset(lo[:], -SCORE_BOUND)
                nc.vector.memset(hi[:], SCORE_BOUND)
                for _ in range(N_BISECT):
                    nc.vector.tensor_tensor(out=mid[:], in0=lo[:], in1=hi[:],
                                            op=ALU.add)
                    nc.vector.tensor_scalar_mul(mid[:], mid[:], 0.5)
                    nc.vector.tensor_scalar(out=ge[:], in0=scores_full[:],
                                            scalar1=mid[:], scalar2=None,
                                            op0=ALU.is_ge)
                    nc.vector.reduce_sum(cnt[:], ge[:],
                                         axis=mybir.AxisListType.X)
                    pc = psm.tile([P, D], dt.float32, tag="psm")
                    nc.tensor.matmul(pc[:, 0:1], ones_pp[:], cnt[:],
                                     start=True, stop=True)
                    nc.vector.tensor_scalar(out=pred[:], in0=pc[:, 0:1],
                                            scalar1=float(K_SEL), scalar2=None,
                                            op0=ALU.is_ge)
                    # lo += pred*(mid-lo); hi += (1-pred)*(mid-hi)
                    nc.vector.tensor_tensor(out=t1[:], in0=mid[:], in1=lo[:],
                                            op=ALU.subtract)
                    nc.vector.tensor_tensor(out=t1[:], in0=t1[:], in1=pred[:],
                                            op=ALU.mult)
                    nc.vector.tensor_tensor(out=lo[:], in0=lo[:], in1=t1[:],
                                            op=ALU.add)
                    nc.vector.tensor_scalar(out=npred[:], in0=pred[:],
                                            scalar1=-1.0, scalar2=1.0,
                                            op0=ALU.mult, op1=ALU.add)
                    nc.vector.tensor_tensor(out=t1[:], in0=mid[:], in1=hi[:],
                                            op=ALU.subtract)
                    nc.vector.tensor_tensor(out=t1[:], in0=t1[:], in1=npred[:],
                                            op=ALU.mult)
                    nc.vector.tensor_tensor(out=hi[:], in0=hi[:], in1=t1[:],
                                            op=ALU.add)
            with nc.named_scope("mask"):
                # selected = score >= thr(=lo); offs = token_id + sel*2^30
                nc.vector.tensor_scalar(out=msel[:], in0=scores_loc[:],
                                        scalar1=lo[:], scalar2=None,
                                        op0=ALU.is_ge)
                nc.gpsimd.iota(ids[:], pattern=[[P, NT]], base=0,
                               channel_multiplier=1)
                nc.vector.tensor_scalar_mul(big_f[:], msel[:], float(2 ** 30))
                nc.vector.tensor_copy(out=big_i[:], in_=big_f[:])
                nc.vector.tensor_tensor(out=offs[:], in0=ids[:], in1=big_i[:],
                                        op=ALU.add)

        # ---- fixup: overwrite pass-through rows with x ----
        with nc.named_scope("fixup"):
            for i in range(NT):
                nc.gpsimd.indirect_dma_start(
                    out=out_d.ap(),
                    out_offset=bass.IndirectOffsetOnAxis(ap=offs[:, i:i + 1],
                                                         axis=0),
                    in_=xns[i][:],
                    in_offset=None,
                    bounds_check=TLOC - 1,
                    oob_is_err=False,
                )

    nc.compile()
    return nc


def _get_nc():
    if "nc" not in _cached:
        _cached["nc"] = build_kernel()
    return _cached["nc"]


def run(x, w_router, w_block, trace=False, trace_kwargs=None):
    nc = _get_nc()
    x = np.ascontiguousarray(x, dtype=np.float32)
    w_router = np.ascontiguousarray(w_router, dtype=np.float32)
    w_block = np.ascontiguousarray(w_block, dtype=np.float32)
    in_maps = []
    for c in range(8):
        b, h = c // 2, c % 2
        in_maps.append({
            "x": x[b, h * TLOC:(h + 1) * TLOC, :],
            "w_router": w_router,
            "w_block": w_block,
        })
    res = run_bass_kernel_spmd(nc, in_maps, core_ids=list(range(8)),
                               trace=trace, **(trace_kwargs or {}))
    out = np.empty((B, L, D), dtype=np.float32)
    for c in range(8):
        b, h = c // 2, c % 2
        out[b, h * TLOC:(h + 1) * TLOC, :] = res.results[c]["out"]
    return out, res


def kernel(x, w_router, w_block):
    out, _ = run(x, w_router, w_block, trace=False)
    return out


# revision 8
# speedup vs baseline: 1.3277x; 1.3277x over previous
"""MoD router kernel for 8 Trainium2 NeuronCores.

Full inputs: x [4, 8192, 1024] f32, w_router [1024] f32, w_block [1024, 1024] f32.
out[b, l] = gelu_tanh(x[b, l] @ w_block) if l in topk(x[b] @ w_router, k=6144)
            else x[b, l]
(top-k membership is all that matters: the reference scatters processed rows
back to their own positions.)

Sharding: core c <- batch row c//2, contiguous half c%2 of L (4096 tokens).
Per core:
  - stream x tiles [128, 1024] f32, keep resident in SBUF
  - scores = x @ w_router on DVE (f32, exact enough to preserve the top-k set)
  - cast x -> bf16, PE-transpose, 16 bf16 matmuls vs w_block (K=1024,
    fp32 PSUM accumulate), tanh-gelu on ACT, bulk-write gelu(xW) for ALL
    tokens to out
  - pairwise AllGather of scores (16KB) -> full-row scores
  - 16-ary search (9 rounds, broadcast-compare against 15 thresholds at once)
    for the k-th largest score; exact: grid points are dyadic, update arith
    reproduces the compared values bit-exactly
  - fixup per tile: indirect-scatter resident f32 x rows over out where
    score < thr (selected rows get OOB offsets -> skipped); overlaps the
    main loop since each scatter only touches its own tile's row range
"""
import sys

if "/opt/trn_rl_repo" not in sys.path:
    sys.path.insert(0, "/opt/trn_rl_repo")

from contextlib import ExitStack

import numpy as np

import concourse.bass as bass
import concourse.tile as tile
from concourse import bacc, mybir
from concourse.bass_utils import run_bass_kernel_spmd
from concourse.masks import make_identity

dt = mybir.dt
AF = mybir.ActivationFunctionType
ALU = mybir.AluOpType

P = 128
B, L, D = 4, 8192, 1024
TLOC = L // 2          # tokens per core
NT = TLOC // P         # 32 t-tiles per core
DC = D // P            # 8 contraction chunks
K_SEL = int(L * 0.75)  # 6144
N_ROUNDS = 9           # 16-ary search: 16^-9 * 128 = 1.9e-9 resolution
SCORE_BOUND = 64.0

_cached = {}


def build_kernel():
    nc = bacc.Bacc("TRN2", target_bir_lowering=False, debug=False, num_devices=8)
    x_d = nc.dram_tensor("x", [TLOC, D], dt.float32, kind="ExternalInput")
    wr_d = nc.dram_tensor("w_router", [D], dt.float32, kind="ExternalInput")
    wb_d = nc.dram_tensor("w_block", [D, D], dt.float32, kind="ExternalInput")
    out_d = nc.dram_tensor("out", [TLOC, D], dt.float32, kind="ExternalOutput")
    sc_in = nc.dram_tensor("sc_in", [TLOC], dt.float32, kind="Internal")
    sc_out = nc.dram_tensor("sc_out", [L], dt.float32, kind="Internal")
    import os
    dbg = os.environ.get("KERNEL_DEBUG") == "1"
    if dbg:
        dbg_sf = nc.dram_tensor("dbg_sf", [P, 2 * NT], dt.float32, kind="ExternalOutput")
        dbg_lo = nc.dram_tensor("dbg_lo", [P, N_ROUNDS], dt.float32, kind="ExternalOutput")
        dbg_cnt = nc.dram_tensor("dbg_cnt", [P, 15 * N_ROUNDS], dt.float32, kind="ExternalOutput")
        dbg_t = nc.dram_tensor("dbg_t", [P, 15 * N_ROUNDS], dt.float32, kind="ExternalOutput")
        dbg_msel = nc.dram_tensor("dbg_msel", [P, NT], dt.float32, kind="ExternalOutput")
        dbg_offs = nc.dram_tensor("dbg_offs", [P, NT], dt.int32, kind="ExternalOutput")

    with tile.TileContext(nc) as tc, ExitStack() as ctx:
        const = ctx.enter_context(tc.tile_pool(name="const", bufs=1))
        xpool = ctx.enter_context(tc.tile_pool(name="xn", bufs=NT))
        wpool = ctx.enter_context(tc.tile_pool(name="wb", bufs=1))
        xbfp = ctx.enter_context(tc.tile_pool(name="xbf", bufs=3))
        xtp = ctx.enter_context(tc.tile_pool(name="xt", bufs=3))
        yp = ctx.enter_context(tc.tile_pool(name="y", bufs=3))
        smalls = ctx.enter_context(tc.tile_pool(name="smalls", bufs=1))
        psx = ctx.enter_context(tc.tile_pool(name="psx", bufs=2, space="PSUM"))
        psy = ctx.enter_context(tc.tile_pool(name="psy", bufs=4, space="PSUM"))

        # ---- constants ----
        ident = const.tile([P, P], dt.bfloat16)
        make_identity(nc, ident[:])
        ones_pp = const.tile([P, P], dt.float32)
        nc.vector.memset(ones_pp[:], 1.0)
        ones_row = const.tile([1, P], dt.float32)
        nc.vector.memset(ones_row[:], 1.0)

        # w_block -> bf16 SBUF [p, dc, e] via SWDGE cast DMA
        w_sb = wpool.tile([P, DC, D], dt.bfloat16)
        nc.gpsimd.dma_start(out=w_sb[:],
                            in_=wb_d.ap().rearrange("(a p) e -> p a e", p=P))

        wr_sb = const.tile([1, D], dt.float32)
        nc.sync.dma_start(wr_sb[:], wr_d.ap())
        # broadcast w_router over all partitions via K=1 matmuls (into psy slots)
        w_rep = const.tile([P, D], dt.float32)
        for h in range(2):
            sl = slice(h * 512, (h + 1) * 512)
            pm = psy.tile([P, 512], dt.float32, tag="psy")
            nc.tensor.matmul(pm[:], ones_row[:], wr_sb[:, sl],
                             start=True, stop=True)
            nc.vector.tensor_copy(w_rep[:, sl], pm[:])

        # ---- score / search tiles ----
        scores_loc = smalls.tile([P, NT], dt.float32)
        scores_full = smalls.tile([P, 2 * NT], dt.float32)
        ge3 = smalls.tile([P, 15, 2 * NT], dt.float32)
        cnts = smalls.tile([P, 15], dt.float32)
        gk = smalls.tile([P, 15], dt.float32)
        tcand = smalls.tile([P, 15], dt.float32)
        jrow_i = smalls.tile([P, 15], dt.int32)
        jrow = smalls.tile([P, 15], dt.float32)
        lo = smalls.tile([P, 1], dt.float32)
        w16 = smalls.tile([P, 1], dt.float32)
        m = smalls.tile([P, 1], dt.float32)
        msel = smalls.tile([P, NT], dt.float32)
        pcol_i = smalls.tile([P, 1], dt.int32)
        pcol = smalls.tile([P, 1], dt.float32)
        offs_f = smalls.tile([P, NT], dt.float32)
        offs = smalls.tile([P, NT], dt.int32)
        tmp = smalls.tile([P, D], dt.float32)

        # ---- main loop: load, score, cast, transpose, GEMM, gelu, write ----
        xns = []
        store_insts = []
        for i in range(NT):
            xn = xpool.tile([P, D], dt.float32, tag="xn")
            xns.append(xn)
            with nc.named_scope("load"):
                nc.sync.dma_start(xn[:], x_d.ap()[i * P:(i + 1) * P, :])
            with nc.named_scope("scores"):
                nc.vector.tensor_tensor(out=tmp[:], in0=xn[:], in1=w_rep[:],
                                        op=ALU.mult)
                nc.vector.reduce_sum(scores_loc[:, i:i + 1], tmp[:],
                                     axis=mybir.AxisListType.X)
            with nc.named_scope("cast"):
                xbf = xbfp.tile([P, D], dt.bfloat16, tag="xbf")
                nc.scalar.copy(xbf[:], xn[:])
            xt = xtp.tile([P, DC, P], dt.bfloat16, tag="xt")
            px = psx.tile([P, DC, P], dt.bfloat16, tag="psx")
            with nc.named_scope("xpose"):
                for c in range(DC):
                    nc.tensor.transpose(px[:, c, :], xbf[:, c * P:(c + 1) * P],
                                        ident[:])
                nc.any.tensor_copy(out=xt[:], in_=px[:])
            y = yp.tile([P, D], dt.float32, tag="y")
            for h in range(2):
                py = psy.tile([P, 512], dt.float32, tag="psy")
                with nc.named_scope("gemm"):
                    for c in range(DC):
                        nc.tensor.matmul(
                            py[:], xt[:, c, :],
                            w_sb[:, c, h * 512:(h + 1) * 512],
                            start=(c == 0), stop=(c == DC - 1))
                with nc.named_scope("gelu"):
                    nc.scalar.activation(y[:, h * 512:(h + 1) * 512], py[:],
                                         AF.Gelu_apprx_tanh)
            with nc.named_scope("store"):
                st = nc.sync.dma_start(out_d.ap()[i * P:(i + 1) * P, :], y[:])
            store_insts.append(st)

        # ---- threshold side-chain (high priority so it never starves) ----
        with tc.high_priority():
            with nc.named_scope("coll"):
                nc.sync.dma_start(sc_in.ap(), scores_loc[:])
                nc.gpsimd.collective_compute(
                    "AllGather", ALU.bypass,
                    ins=[sc_in.ap()], outs=[sc_out.ap()],
                    replica_groups=[[0, 1], [2, 3], [4, 5], [6, 7]])
                nc.sync.dma_start(scores_full[:], sc_out.ap())
            with nc.named_scope("search"):
                # jrow = 1..15 replicated on every partition
                nc.gpsimd.iota(jrow_i[:], pattern=[[1, 15]], base=1,
                               channel_multiplier=0)
                nc.vector.tensor_copy(out=jrow[:], in_=jrow_i[:])
                nc.vector.memset(lo[:], -SCORE_BOUND)
                nc.vector.memset(w16[:], 2.0 * SCORE_BOUND / 16.0)
                if dbg:
                    nc.sync.dma_start(dbg_sf.ap(), scores_full[:])
                sc_b = scores_full[:].rearrange("p (a x) -> p a x", a=1) \
                    .to_broadcast([P, 15, 2 * NT])
                t_b = tcand[:].rearrange("p (j x) -> p j x", x=1) \
                    .to_broadcast([P, 15, 2 * NT])
                for _r in range(N_ROUNDS):
                    # tcand[:, j] = lo + (j+1)*w16  (dyadic, exact fp32)
                    nc.vector.tensor_scalar(out=tcand[:], in0=jrow[:],
                                            scalar1=w16[:], scalar2=lo[:],
                                            op0=ALU.mult, op1=ALU.add)
                    nc.vector.tensor_tensor(out=ge3[:], in0=sc_b, in1=t_b,
                                            op=ALU.is_ge)
                    nc.vector.reduce_sum(cnts[:], ge3[:],
                                         axis=mybir.AxisListType.X)
                    pc = psy.tile([P, 512], dt.float32, tag="psy")
                    nc.tensor.matmul(pc[:, :15], ones_pp[:], cnts[:],
                                     start=True, stop=True)
                    nc.vector.tensor_scalar(out=gk[:], in0=pc[:, :15],
                                            scalar1=float(K_SEL), scalar2=None,
                                            op0=ALU.is_ge)
                    nc.vector.reduce_sum(m[:], gk[:],
                                         axis=mybir.AxisListType.X)
                    # lo += m*w16 (same dyadic arithmetic as tcand)
                    nc.vector.tensor_scalar(out=lo[:], in0=m[:],
                                            scalar1=w16[:], scalar2=lo[:],
                                            op0=ALU.mult, op1=ALU.add)
                    nc.vector.tensor_scalar_mul(w16[:], w16[:], 1.0 / 16.0)
                    if dbg:
                        nc.sync.dma_start(dbg_lo.ap()[:, _r:_r + 1], lo[:])
                        dbc = smalls.tile([P, 15], dt.float32, tag=f"dbc{_r}")
                        nc.vector.tensor_copy(out=dbc[:], in_=pc[:, :15])
                        nc.sync.dma_start(
                            dbg_cnt.ap()[:, _r * 15:(_r + 1) * 15], dbc[:])
                        nc.sync.dma_start(
                            dbg_t.ap()[:, _r * 15:(_r + 1) * 15], tcand[:])
            with nc.named_scope("mask"):
                # selected = score >= thr(=lo); offs = p + sel*2^30 (rel.)
                nc.vector.tensor_scalar(out=msel[:], in0=scores_loc[:],
                                        scalar1=lo[:], scalar2=None,
                                        op0=ALU.is_ge)
                nc.gpsimd.iota(pcol_i[:], pattern=[[0, 1]], base=0,
                               channel_multiplier=1)
                nc.vector.tensor_copy(out=pcol[:], in_=pcol_i[:])
                nc.vector.tensor_scalar(out=offs_f[:], in0=msel[:],
                                        scalar1=float(2 ** 30),
                                        scalar2=pcol[:],
                                        op0=ALU.mult, op1=ALU.add)
                nc.vector.tensor_copy(out=offs[:], in_=offs_f[:])
                if dbg:
                    nc.sync.dma_start(dbg_msel.ap(), msel[:])
                    nc.sync.dma_start(dbg_offs.ap(), offs[:])

        # ---- fixup: overwrite pass-through rows with resident x rows.
        # Placed after the mask in program order so the RAW dep on offs is
        # tracked; each scatter additionally depends only on its own tile's
        # store, so scatters overlap the tail of the main loop.
        with nc.named_scope("fixup"):
            for i in range(NT):
                sl = out_d.ap()[i * P:(i + 1) * P, :]
                sl_rel = bass.AP(tensor=sl.tensor, offset=0, ap=sl.ap,
                                 dep_tracking_offset=i * P * D)
                fx = nc.gpsimd.indirect_dma_start(
                    out=sl_rel,
                    out_offset=bass.IndirectOffsetOnAxis(ap=offs[:, i:i + 1],
                                                         axis=0),
                    in_=xns[i][:],
                    in_offset=None,
                    element_offset=i * P * D,
                    bounds_check=P - 1,
                    oob_is_err=False,
                )
                tile.add_dep_helper(fx.ins, store_insts[i].ins,
                                    reason="fixup scatter after bulk y store")

    nc.compile()
    return nc


def _get_nc():
    if "nc" not in _cached:
        _cached["nc"] = build_kernel()
    return _cached["nc"]


def run(x, w_router, w_block, trace=False, trace_kwargs=None):
    nc = _get_nc()
    x = np.ascontiguousarray(x, dtype=np.float32)
    w_router = np.ascontiguousarray(w_router, dtype=np.float32)
    w_block = np.ascontiguousarray(w_block, dtype=np.float32)
    in_maps = []
    for c in range(8):
        b, h = c // 2, c % 2
        in_maps.append({
            "x": x[b, h * TLOC:(h + 1) * TLOC, :],
            "w_router": w_router,
            "w_block": w_block,
        })
    res = run_bass_kernel_spmd(nc, in_maps, core_ids=list(range(8)),
                               trace=trace, **(trace_kwargs or {}))
    out = np.empty((B, L, D), dtype=np.float32)
    for c in range(8):
        b, h = c // 2, c % 2
        out[b, h * TLOC:(h + 1) * TLOC, :] = res.results[c]["out"]
    return out, res


def kernel(x, w_router, w_block):
    out, _ = run(x, w_router, w_block, trace=False)
    return out


# revision 12
# speedup vs baseline: 1.3304x; 1.0020x over previous
"""MoD router kernel for 8 Trainium2 NeuronCores.

Full inputs: x [4, 8192, 1024] f32, w_router [1024] f32, w_block [1024, 1024] f32.
out[b, l] = gelu_tanh(x[b, l] @ w_block) if l in topk(x[b] @ w_router, k=6144)
            else x[b, l]
(top-k membership is all that matters: the reference scatters processed rows
back to their own positions.)

Sharding: core c <- batch row c//2, contiguous half c%2 of L (4096 tokens).
Per core:
  - front-loaded streaming of x (8x 2MiB DMAs), x stays resident in SBUF
  - scores = x @ w_router on DVE (f32; preserves the exact top-k set)
  - cast x -> bf16 (DVE), PE-transpose (bf16, 1cyc/row), 16 bf16 matmuls vs
    w_block (K=1024, fp32 PSUM), tanh-gelu on ACT, bulk-write gelu(xW) for
    ALL tokens
  - pairwise AllGather of scores (16KB) -> full-row scores
  - 16-ary threshold search, 9 rounds: one broadcast-compare [P,15,64] +
    reduce per round, cross-partition count via a tiny PE matmul, fused
    interval-select via tensor_scalar accum_out. Grid points are dyadic and
    recomputed bit-identically, so the final lo is exactly the k-th largest.
  - fixup: per-tile indirect scatter overwrites pass-through rows with the
    resident f32 x rows (selected rows get OOB offsets -> skipped); each
    scatter depends only on its tile's store, so the tail overlaps.
"""
import sys

if "/opt/trn_rl_repo" not in sys.path:
    sys.path.insert(0, "/opt/trn_rl_repo")

import os
from contextlib import ExitStack

import numpy as np

import concourse.bass as bass
import concourse.tile as tile
from concourse import bacc, mybir
from concourse.bass_utils import run_bass_kernel_spmd
from concourse.masks import make_identity

dt = mybir.dt
AF = mybir.ActivationFunctionType
ALU = mybir.AluOpType

P = 128
B, L, D = 4, 8192, 1024
TLOC = L // 2          # tokens per core
NT = TLOC // P         # 32 t-tiles per core
DC = D // P            # 8 contraction chunks
K_SEL = int(L * 0.75)  # 6144
N_ROUNDS = 9           # 16^-9 * 128 = 1.9e-9 resolution
SCORE_BOUND = 64.0
LOAD_CHUNK = 4         # t-tiles per load DMA (2 MiB)

_cached = {}


def build_kernel():
    nc = bacc.Bacc("TRN2", target_bir_lowering=False, debug=False, num_devices=8)
    x_d = nc.dram_tensor("x", [TLOC, D], dt.float32, kind="ExternalInput")
    wr_d = nc.dram_tensor("w_router", [D], dt.float32, kind="ExternalInput")
    wb_d = nc.dram_tensor("w_block", [D, D], dt.float32, kind="ExternalInput")
    out_d = nc.dram_tensor("out", [TLOC, D], dt.float32, kind="ExternalOutput")
    sc_in = nc.dram_tensor("sc_in", [TLOC], dt.float32, kind="Internal")
    sc_out = nc.dram_tensor("sc_out", [L], dt.float32, kind="Internal")
    dbg = os.environ.get("KERNEL_DEBUG") == "1"
    if dbg:
        dbg_lo = nc.dram_tensor("dbg_lo", [P, N_ROUNDS], dt.float32,
                                kind="ExternalOutput")
        dbg_msel = nc.dram_tensor("dbg_msel", [P, NT], dt.float32,
                                  kind="ExternalOutput")

    with tile.TileContext(nc) as tc, ExitStack() as ctx:
        const = ctx.enter_context(tc.tile_pool(name="const", bufs=1))
        xpool = ctx.enter_context(tc.tile_pool(name="xn", bufs=1))
        wpool = ctx.enter_context(tc.tile_pool(name="wb", bufs=1))
        xbfp = ctx.enter_context(tc.tile_pool(name="xbf", bufs=3))
        xtp = ctx.enter_context(tc.tile_pool(name="xt", bufs=3))
        yp = ctx.enter_context(tc.tile_pool(name="y", bufs=3))
        smalls = ctx.enter_context(tc.tile_pool(name="smalls", bufs=1))
        psx = ctx.enter_context(tc.tile_pool(name="psx", bufs=2, space="PSUM"))
        psy = ctx.enter_context(tc.tile_pool(name="psy", bufs=5, space="PSUM"))

        # ---- constants ----
        ident = const.tile([P, P], dt.bfloat16)
        make_identity(nc, ident[:])
        ones_pp = const.tile([P, P], dt.float32)
        nc.vector.memset(ones_pp[:], 1.0)
        ones_row = const.tile([1, P], dt.float32)
        nc.vector.memset(ones_row[:], 1.0)

        # w_block -> bf16 SBUF [p, dc, e]; per-chunk casts so the first GEMM
        # matmuls don't wait for the whole 4 MiB
        w_sb = wpool.tile([P, DC, D], dt.bfloat16)
        for c in range(DC):
            nc.gpsimd.dma_start(out=w_sb[:, c, :],
                                in_=wb_d.ap()[c * P:(c + 1) * P, :])

        wr_sb = const.tile([1, D], dt.float32)
        nc.sync.dma_start(wr_sb[:], wr_d.ap())
        # broadcast w_router over all partitions via K=1 matmuls
        w_rep = const.tile([P, D], dt.float32)
        for h in range(2):
            sl = slice(h * 512, (h + 1) * 512)
            pm = psy.tile([P, 512], dt.float32, tag="psy")
            nc.tensor.matmul(pm[:], ones_row[:], wr_sb[:, sl],
                             start=True, stop=True)
            nc.vector.tensor_copy(w_rep[:, sl], pm[:])

        # ---- score / search tiles ----
        scores_loc = smalls.tile([P, NT], dt.float32)
        scores_full = smalls.tile([P, 2 * NT], dt.float32)
        ge3 = smalls.tile([P, 15, 2 * NT], dt.float32)
        cnts = smalls.tile([P, 15], dt.float32)
        gk = smalls.tile([P, 15], dt.float32)
        tcand = smalls.tile([P, 15], dt.float32)
        jrow_i = smalls.tile([P, 15], dt.int32)
        jrow = smalls.tile([P, 15], dt.float32)
        lo = smalls.tile([P, 1], dt.float32)
        w16t = smalls.tile([P, 1], dt.float32)
        m = smalls.tile([P, 1], dt.float32)
        msel = smalls.tile([P, NT], dt.float32)
        pcol_i = smalls.tile([P, 1], dt.int32)
        pcol = smalls.tile([P, 1], dt.float32)
        offs_f = smalls.tile([P, NT], dt.float32)
        offs = smalls.tile([P, NT], dt.int32)
        tmp = smalls.tile([P, D], dt.float32)

        # ---- front-loaded streaming + scores ----
        xn_all = xpool.tile([P, NT, D], dt.float32)
        for j in range(NT // LOAD_CHUNK):
            a = j * LOAD_CHUNK
            with nc.named_scope("load"):
                nc.sync.dma_start(
                    xn_all[:, a:a + LOAD_CHUNK, :],
                    x_d.ap()[a * P:(a + LOAD_CHUNK) * P, :].rearrange(
                        "(c p) d -> p c d", p=P))
            with nc.named_scope("scores"):
                for i in range(a, a + LOAD_CHUNK):
                    nc.vector.tensor_tensor(out=tmp[:], in0=xn_all[:, i, :],
                                            in1=w_rep[:], op=ALU.mult)
                    nc.vector.reduce_sum(scores_loc[:, i:i + 1], tmp[:],
                                         axis=mybir.AxisListType.X)

        # ---- main compute loop ----
        store_insts = []
        for i in range(NT):
            with nc.named_scope("cast"):
                xbf = xbfp.tile([P, D], dt.bfloat16, tag="xbf")
                nc.vector.tensor_copy(out=xbf[:], in_=xn_all[:, i, :])
            xt = xtp.tile([P, DC, P], dt.bfloat16, tag="xt")
            px = psx.tile([P, DC, P], dt.bfloat16, tag="psx")
            with nc.named_scope("xpose"):
                for c in range(DC):
                    nc.tensor.transpose(px[:, c, :], xbf[:, c * P:(c + 1) * P],
                                        ident[:])
                nc.scalar.copy(xt[:], px[:])
            y = yp.tile([P, D], dt.float32, tag="y")
            for h in range(2):
                py = psy.tile([P, 512], dt.float32, tag="psy")
                with nc.named_scope("gemm"):
                    for c in range(DC):
                        nc.tensor.matmul(
                            py[:], xt[:, c, :],
                            w_sb[:, c, h * 512:(h + 1) * 512],
                            start=(c == 0), stop=(c == DC - 1))
                with nc.named_scope("gelu"):
                    nc.scalar.activation(y[:, h * 512:(h + 1) * 512], py[:],
                                         AF.Gelu_apprx_tanh)
            with nc.named_scope("store"):
                st = nc.sync.dma_start(out_d.ap()[i * P:(i + 1) * P, :], y[:])
            store_insts.append(st)

        # ---- threshold side-chain (high priority so it never starves) ----
        with tc.high_priority():
            with nc.named_scope("coll"):
                nc.sync.dma_start(sc_in.ap(), scores_loc[:])
                nc.gpsimd.collective_compute(
                    "AllGather", ALU.bypass,
                    ins=[sc_in.ap()], outs=[sc_out.ap()],
                    replica_groups=[[0, 1], [2, 3], [4, 5], [6, 7]])
                nc.sync.dma_start(scores_full[:], sc_out.ap())
            with nc.named_scope("search"):
                # jrow = 1..15 replicated on every partition
                nc.gpsimd.iota(jrow_i[:], pattern=[[1, 15]], base=1,
                               channel_multiplier=0)
                nc.vector.tensor_copy(out=jrow[:], in_=jrow_i[:])
                nc.vector.memset(lo[:], -SCORE_BOUND)
                nc.vector.memset(w16t[:], 2.0 * SCORE_BOUND / 16.0)
                sc_b = scores_full[:].rearrange("p (a x) -> p a x", a=1) \
                    .to_broadcast([P, 15, 2 * NT])
                t_b = tcand[:].rearrange("p (j x) -> p j x", x=1) \
                    .to_broadcast([P, 15, 2 * NT])
                for r in range(N_ROUNDS):
                    # tcand[:, j] = lo + (j+1)*w16  (dyadic, exact fp32)
                    nc.vector.tensor_scalar(out=tcand[:], in0=jrow[:],
                                            scalar1=w16t[:], scalar2=lo[:],
                                            op0=ALU.mult, op1=ALU.add)
                    nc.vector.tensor_tensor(out=ge3[:], in0=sc_b, in1=t_b,
                                            op=ALU.is_ge)
                    nc.vector.reduce_sum(cnts[:], ge3[:],
                                         axis=mybir.AxisListType.X)
                    pc = psy.tile([P, 512], dt.float32, tag="psy")
                    nc.tensor.matmul(pc[:, :15], ones_pp[:], cnts[:],
                                     start=True, stop=True)
                    # gk = (count >= k); m = #intervals passed (row-sum)
                    nc.vector.tensor_scalar(out=gk[:], in0=pc[:, :15],
                                            scalar1=float(K_SEL), scalar2=None,
                                            op0=ALU.is_ge)
                    nc.vector.reduce_sum(m[:], gk[:],
                                         axis=mybir.AxisListType.X)
                    # lo += m*w16 (bit-identical to the compared grid point)
                    nc.vector.tensor_scalar(out=lo[:], in0=m[:],
                                            scalar1=w16t[:], scalar2=lo[:],
                                            op0=ALU.mult, op1=ALU.add)
                    nc.vector.tensor_scalar_mul(w16t[:], w16t[:], 1.0 / 16.0)
                    if dbg:
                        nc.sync.dma_start(dbg_lo.ap()[:, r:r + 1], lo[:])
            with nc.named_scope("mask"):
                # selected = score >= thr(=lo); offs = p + sel*2^30 (per-tile)
                nc.vector.tensor_scalar(out=msel[:], in0=scores_loc[:],
                                        scalar1=lo[:], scalar2=None,
                                        op0=ALU.is_ge)
                nc.gpsimd.iota(pcol_i[:], pattern=[[0, 1]], base=0,
                               channel_multiplier=1)
                nc.vector.tensor_copy(out=pcol[:], in_=pcol_i[:])
                nc.vector.tensor_scalar(out=offs_f[:], in0=msel[:],
                                        scalar1=float(2 ** 30),
                                        scalar2=pcol[:],
                                        op0=ALU.mult, op1=ALU.add)
                nc.vector.tensor_copy(out=offs[:], in_=offs_f[:])
                if dbg:
                    nc.sync.dma_start(dbg_msel.ap(), msel[:])

        # ---- fixup: overwrite pass-through rows with resident x rows ----
        with nc.named_scope("fixup"):
            for i in range(NT):
                sl = out_d.ap()[i * P:(i + 1) * P, :]
                sl_rel = bass.AP(tensor=sl.tensor, offset=0, ap=sl.ap,
                                 dep_tracking_offset=i * P * D)
                fx = nc.gpsimd.indirect_dma_start(
                    out=sl_rel,
                    out_offset=bass.IndirectOffsetOnAxis(ap=offs[:, i:i + 1],
                                                         axis=0),
                    in_=xn_all[:, i, :],
                    in_offset=None,
                    element_offset=i * P * D,
                    bounds_check=P - 1,
                    oob_is_err=False,
                )
                tile.add_dep_helper(fx.ins, store_insts[i].ins,
                                    reason="fixup scatter after bulk y store")

    nc.compile()
    return nc


def _get_nc():
    if "nc" not in _cached:
        _cached["nc"] = build_kernel()
    return _cached["nc"]


def run(x, w_router, w_block, trace=False, trace_kwargs=None):
    nc = _get_nc()
    x = np.ascontiguousarray(x, dtype=np.float32)
    w_router = np.ascontiguousarray(w_router, dtype=np.float32)
    w_block = np.ascontiguousarray(w_block, dtype=np.float32)
    in_maps = []
    for c in range(8):
        b, h = c // 2, c % 2
        in_maps.append({
            "x": x[b, h * TLOC:(h + 1) * TLOC, :],
            "w_router": w_router,
            "w_block": w_block,
        })
    res = run_bass_kernel_spmd(nc, in_maps, core_ids=list(range(8)),
                               trace=trace, **(trace_kwargs or {}))
    out = np.empty((B, L, D), dtype=np.float32)
    for c in range(8):
        b, h = c // 2, c % 2
        out[b, h * TLOC:(h + 1) * TLOC, :] = res.results[c]["out"]
    return out, res


def kernel(x, w_router, w_block):
    out, _ = run(x, w_router, w_block, trace=False)
    return out


# revision 13
# speedup vs baseline: 1.3316x; 1.0009x over previous
"""MoD router kernel for 8 Trainium2 NeuronCores.

Full inputs: x [4, 8192, 1024] f32, w_router [1024] f32, w_block [1024, 1024] f32.
out[b, l] = gelu_tanh(x[b, l] @ w_block) if l in topk(x[b] @ w_router, k=6144)
            else x[b, l]
(top-k membership is all that matters: the reference scatters processed rows
back to their own positions.)

Sharding: core c <- batch row c//2, contiguous half c%2 of L (4096 tokens).
Per core:
  - front-loaded streaming of x (8x 2MiB DMAs), x stays resident in SBUF
  - scores = x @ w_router on DVE (f32; preserves the exact top-k set)
  - cast x -> bf16 (DVE), PE-transpose (bf16, 1cyc/row), 16 bf16 matmuls vs
    w_block (K=1024, fp32 PSUM), tanh-gelu on ACT, bulk-write gelu(xW) for
    ALL tokens
  - pairwise AllGather of scores (16KB) -> full-row scores
  - 16-ary threshold search, 9 rounds: one broadcast-compare [P,15,64] +
    reduce per round, cross-partition count via a tiny PE matmul, fused
    interval-select via tensor_scalar accum_out. Grid points are dyadic and
    recomputed bit-identically, so the final lo is exactly the k-th largest.
  - fixup: per-tile indirect scatter overwrites pass-through rows with the
    resident f32 x rows (selected rows get OOB offsets -> skipped); each
    scatter depends only on its tile's store, so the tail overlaps.
"""
import sys

if "/opt/trn_rl_repo" not in sys.path:
    sys.path.insert(0, "/opt/trn_rl_repo")

import os
from contextlib import ExitStack

import numpy as np

import concourse.bass as bass
import concourse.tile as tile
from concourse import bacc, mybir
from concourse.bass_utils import run_bass_kernel_spmd
from concourse.masks import make_identity

dt = mybir.dt
AF = mybir.ActivationFunctionType
ALU = mybir.AluOpType

P = 128
B, L, D = 4, 8192, 1024
TLOC = L // 2          # tokens per core
NT = TLOC // P         # 32 t-tiles per core
DC = D // P            # 8 contraction chunks
K_SEL = int(L * 0.75)  # 6144
N_ROUNDS = 9           # 16^-9 * 128 = 1.9e-9 resolution
SCORE_BOUND = 64.0
LOAD_CHUNK = 4         # t-tiles per load DMA (2 MiB)

_cached = {}


def build_kernel():
    nc = bacc.Bacc("TRN2", target_bir_lowering=False, debug=False, num_devices=8)
    x_d = nc.dram_tensor("x", [TLOC, D], dt.float32, kind="ExternalInput")
    wr_d = nc.dram_tensor("w_router", [D], dt.float32, kind="ExternalInput")
    wb_d = nc.dram_tensor("w_block", [D, D], dt.float32, kind="ExternalInput")
    out_d = nc.dram_tensor("out", [TLOC, D], dt.float32, kind="ExternalOutput")
    sc_in = nc.dram_tensor("sc_in", [TLOC], dt.float32, kind="Internal")
    sc_out = nc.dram_tensor("sc_out", [L], dt.float32, kind="Internal")
    dbg = os.environ.get("KERNEL_DEBUG") == "1"
    if dbg:
        dbg_lo = nc.dram_tensor("dbg_lo", [P, N_ROUNDS], dt.float32,
                                kind="ExternalOutput")
        dbg_msel = nc.dram_tensor("dbg_msel", [P, NT], dt.float32,
                                  kind="ExternalOutput")

    with tile.TileContext(nc) as tc, ExitStack() as ctx:
        const = ctx.enter_context(tc.tile_pool(name="const", bufs=1))
        xpool = ctx.enter_context(tc.tile_pool(name="xn", bufs=1))
        wpool = ctx.enter_context(tc.tile_pool(name="wb", bufs=1))
        xbfp = ctx.enter_context(tc.tile_pool(name="xbf", bufs=4))
        xtp = ctx.enter_context(tc.tile_pool(name="xt", bufs=4))
        yp = ctx.enter_context(tc.tile_pool(name="y", bufs=2))
        smalls = ctx.enter_context(tc.tile_pool(name="smalls", bufs=1))
        psx = ctx.enter_context(tc.tile_pool(name="psx", bufs=2, space="PSUM"))
        psy = ctx.enter_context(tc.tile_pool(name="psy", bufs=3, space="PSUM"))

        # ---- constants ----
        ident = const.tile([P, P], dt.bfloat16)
        make_identity(nc, ident[:])
        ones_pp = const.tile([P, P], dt.float32)
        nc.vector.memset(ones_pp[:], 1.0)
        ones_row = const.tile([1, P], dt.float32)
        nc.vector.memset(ones_row[:], 1.0)

        # w_block -> bf16 SBUF [p, dc, e]. HWDGE f32 staging + ACT cast:
        # keeps gpsimd (SWDGE) free so the collective trigger fires early.
        w_sb = wpool.tile([P, DC, D], dt.bfloat16)
        for c in range(DC):
            wstage = wpool.tile([P, D], dt.float32, tag="wstage", bufs=2)
            nc.sync.dma_start(wstage[:], wb_d.ap()[c * P:(c + 1) * P, :])
            nc.scalar.copy(w_sb[:, c, :], wstage[:])

        wr_sb = const.tile([1, D], dt.float32)
        nc.sync.dma_start(wr_sb[:], wr_d.ap())
        # broadcast w_router over all partitions via K=1 matmuls
        w_rep = const.tile([P, D], dt.float32)
        for h in range(2):
            sl = slice(h * 512, (h + 1) * 512)
            pm = psy.tile([P, D], dt.float32, tag="psy")
            nc.tensor.matmul(pm[:, :512], ones_row[:], wr_sb[:, sl],
                             start=True, stop=True)
            nc.vector.tensor_copy(w_rep[:, sl], pm[:, :512])

        # ---- score / search tiles ----
        scores_loc = smalls.tile([P, NT], dt.float32)
        scores_full = smalls.tile([P, 2 * NT], dt.float32)
        ge3 = smalls.tile([P, 15, 2 * NT], dt.float32)
        cnts = smalls.tile([P, 15], dt.float32)
        gk = smalls.tile([P, 15], dt.float32)
        tcand = smalls.tile([P, 15], dt.float32)
        jrow_i = smalls.tile([P, 15], dt.int32)
        jrow = smalls.tile([P, 15], dt.float32)
        lo = smalls.tile([P, 1], dt.float32)
        w16t = smalls.tile([P, 1], dt.float32)
        m = smalls.tile([P, 1], dt.float32)
        msel = smalls.tile([P, NT], dt.float32)
        pcol_i = smalls.tile([P, 1], dt.int32)
        pcol = smalls.tile([P, 1], dt.float32)
        offs_f = smalls.tile([P, NT], dt.float32)
        offs = smalls.tile([P, NT], dt.int32)
        tmp = smalls.tile([P, D], dt.float32)

        # ---- front-loaded streaming + scores ----
        xn_all = xpool.tile([P, NT, D], dt.float32)
        for j in range(NT // LOAD_CHUNK):
            a = j * LOAD_CHUNK
            with nc.named_scope("load"):
                nc.sync.dma_start(
                    xn_all[:, a:a + LOAD_CHUNK, :],
                    x_d.ap()[a * P:(a + LOAD_CHUNK) * P, :].rearrange(
                        "(c p) d -> p c d", p=P))
            with nc.named_scope("scores"):
                for i in range(a, a + LOAD_CHUNK):
                    nc.vector.tensor_tensor(out=tmp[:], in0=xn_all[:, i, :],
                                            in1=w_rep[:], op=ALU.mult)
                    nc.vector.reduce_sum(scores_loc[:, i:i + 1], tmp[:],
                                         axis=mybir.AxisListType.X)

        # ---- main compute loop ----
        store_insts = []
        for i in range(NT):
            with nc.named_scope("cast"):
                xbf = xbfp.tile([P, D], dt.bfloat16, tag="xbf")
                nc.vector.tensor_copy(out=xbf[:], in_=xn_all[:, i, :])
            xt = xtp.tile([P, DC, P], dt.bfloat16, tag="xt")
            px = psx.tile([P, DC, P], dt.bfloat16, tag="psx")
            with nc.named_scope("xpose"):
                for c in range(DC):
                    nc.tensor.transpose(px[:, c, :], xbf[:, c * P:(c + 1) * P],
                                        ident[:])
                nc.vector.tensor_copy(out=xt[:], in_=px[:])
            y = yp.tile([P, D], dt.float32, tag="y")
            py = psy.tile([P, D], dt.float32, tag="psy")
            with nc.named_scope("gemm"):
                for h in range(2):
                    for c in range(DC):
                        nc.tensor.matmul(
                            py[:, h * 512:(h + 1) * 512], xt[:, c, :],
                            w_sb[:, c, h * 512:(h + 1) * 512],
                            start=(c == 0), stop=(c == DC - 1))
            with nc.named_scope("gelu"):
                nc.scalar.activation(y[:], py[:], AF.Gelu_apprx_tanh)
            with nc.named_scope("store"):
                st = nc.sync.dma_start(out_d.ap()[i * P:(i + 1) * P, :], y[:])
            store_insts.append(st)

        # ---- threshold side-chain (high priority so it never starves) ----
        with tc.high_priority():
            with nc.named_scope("coll"):
                nc.sync.dma_start(sc_in.ap(), scores_loc[:])
                nc.gpsimd.collective_compute(
                    "AllGather", ALU.bypass,
                    ins=[sc_in.ap()], outs=[sc_out.ap()],
                    replica_groups=[[0, 1], [2, 3], [4, 5], [6, 7]])
                nc.sync.dma_start(scores_full[:], sc_out.ap())
            with nc.named_scope("search"):
                # jrow = 1..15 replicated on every partition
                nc.gpsimd.iota(jrow_i[:], pattern=[[1, 15]], base=1,
                               channel_multiplier=0)
                nc.vector.tensor_copy(out=jrow[:], in_=jrow_i[:])
                nc.vector.memset(lo[:], -SCORE_BOUND)
                nc.vector.memset(w16t[:], 2.0 * SCORE_BOUND / 16.0)
                sc_b = scores_full[:].rearrange("p (a x) -> p a x", a=1) \
                    .to_broadcast([P, 15, 2 * NT])
                t_b = tcand[:].rearrange("p (j x) -> p j x", x=1) \
                    .to_broadcast([P, 15, 2 * NT])
                for r in range(N_ROUNDS):
                    # tcand[:, j] = lo + (j+1)*w16  (dyadic, exact fp32)
                    nc.vector.tensor_scalar(out=tcand[:], in0=jrow[:],
                                            scalar1=w16t[:], scalar2=lo[:],
                                            op0=ALU.mult, op1=ALU.add)
                    nc.vector.tensor_tensor(out=ge3[:], in0=sc_b, in1=t_b,
                                            op=ALU.is_ge)
                    nc.vector.reduce_sum(cnts[:], ge3[:],
                                         axis=mybir.AxisListType.X)
                    pc = psy.tile([P, D], dt.float32, tag="psy")
                    nc.tensor.matmul(pc[:, :15], ones_pp[:], cnts[:],
                                     start=True, stop=True)
                    # gk = (count >= k); m = #intervals passed (row-sum)
                    nc.vector.tensor_scalar(out=gk[:], in0=pc[:, :15],
                                            scalar1=float(K_SEL), scalar2=None,
                                            op0=ALU.is_ge)
                    nc.vector.reduce_sum(m[:], gk[:],
                                         axis=mybir.AxisListType.X)
                    # lo += m*w16 (bit-identical to the compared grid point)
                    nc.vector.tensor_scalar(out=lo[:], in0=m[:],
                                            scalar1=w16t[:], scalar2=lo[:],
                                            op0=ALU.mult, op1=ALU.add)
                    nc.vector.tensor_scalar_mul(w16t[:], w16t[:], 1.0 / 16.0)
                    if dbg:
                        nc.sync.dma_start(dbg_lo.ap()[:, r:r + 1], lo[:])
            with nc.named_scope("mask"):
                # selected = score >= thr(=lo); offs = p + sel*2^30 (per-tile)
                nc.vector.tensor_scalar(out=msel[:], in0=scores_loc[:],
                                        scalar1=lo[:], scalar2=None,
                                        op0=ALU.is_ge)
                nc.gpsimd.iota(pcol_i[:], pattern=[[0, 1]], base=0,
                               channel_multiplier=1)
                nc.vector.tensor_copy(out=pcol[:], in_=pcol_i[:])
                nc.vector.tensor_scalar(out=offs_f[:], in0=msel[:],
                                        scalar1=float(2 ** 30),
                                        scalar2=pcol[:],
                                        op0=ALU.mult, op1=ALU.add)
                nc.vector.tensor_copy(out=offs[:], in_=offs_f[:])
                if dbg:
                    nc.sync.dma_start(dbg_msel.ap(), msel[:])

        # ---- fixup: overwrite pass-through rows with resident x rows ----
        with nc.named_scope("fixup"):
            for i in range(NT):
                sl = out_d.ap()[i * P:(i + 1) * P, :]
                sl_rel = bass.AP(tensor=sl.tensor, offset=0, ap=sl.ap,
                                 dep_tracking_offset=i * P * D)
                fx = nc.gpsimd.indirect_dma_start(
                    out=sl_rel,
                    out_offset=bass.IndirectOffsetOnAxis(ap=offs[:, i:i + 1],
                                                         axis=0),
                    in_=xn_all[:, i, :],
                    in_offset=None,
                    element_offset=i * P * D,
                    bounds_check=P - 1,
                    oob_is_err=False,
                )
                tile.add_dep_helper(fx.ins, store_insts[i].ins,
                                    reason="fixup scatter after bulk y store")

    nc.compile()
    return nc


def _get_nc():
    if "nc" not in _cached:
        _cached["nc"] = build_kernel()
    return _cached["nc"]


def run(x, w_router, w_block, trace=False, trace_kwargs=None):
    nc = _get_nc()
    x = np.ascontiguousarray(x, dtype=np.float32)
    w_router = np.ascontiguousarray(w_router, dtype=np.float32)
    w_block = np.ascontiguousarray(w_block, dtype=np.float32)
    in_maps = []
    for c in range(8):
        b, h = c // 2, c % 2
        in_maps.append({
            "x": x[b, h * TLOC:(h + 1) * TLOC, :],
            "w_router": w_router,
            "w_block": w_block,
        })
    res = run_bass_kernel_spmd(nc, in_maps, core_ids=list(range(8)),
                               trace=trace, **(trace_kwargs or {}))
    out = np.empty((B, L, D), dtype=np.float32)
    for c in range(8):
        b, h = c // 2, c % 2
        out[b, h * TLOC:(h + 1) * TLOC, :] = res.results[c]["out"]
    return out, res


def kernel(x, w_router, w_block):
    out, _ = run(x, w_router, w_block, trace=False)
    return out
